# revision 4
# baseline (speedup 1.0000x reference)
"""MoE layer (B=2, S=2048, D=1024, F=4096, E=8, top-2) on 8 Trainium2 NeuronCores.

Strategy: expert-parallel DENSE. Core c holds expert c's weights and computes
expert c's SwiGLU output for ALL 4096 tokens (perfect load balance regardless of
routing skew). The gate network runs fp32 (exact routing), sharded over tokens
(512 per core); per-expert combine weights (zero except each token's top-2) are
exchanged with a tiny AllToAll, each core scales its expert output rows by its
weight column, and a ReduceScatter sums the 8 weighted expert outputs and leaves
each core with its 512-token shard of the final output. The host concatenates
the 8 shards. Expert matmuls run in float32r (TF32-like, ~1.5e-4 rel err, full
PE rate); the gate runs in true fp32 (4x slower, but tiny).
"""
import numpy as np
from contextlib import ExitStack

import concourse.bass as bass
from concourse import bacc
import concourse.mybir as mybir
import concourse.tile as tile
import concourse.bass_utils as bass_utils
from concourse.masks import make_identity

P = 128
D, F, E, HID = 1024, 4096, 8, 512
NCORES = 8
B, S = 2, 2048
N = B * S            # 4096 tokens
TB = 512             # tokens per block == tokens per core gate-slice
NB = N // TB         # 8
DC = D // P          # 8 contraction chunks over D
NFG = F // 512       # 8 f-groups of 512
NTS = TB // P        # 4 token subtiles
NHT = HID // P       # 4 hidden tiles (gate)
FP32 = mybir.dt.float32
F32R = mybir.dt.float32r
AX = mybir.AxisListType
OP = mybir.AluOpType
ACTF = mybir.ActivationFunctionType
RG = [list(range(NCORES))]

_CACHED_NC = None


def build():
    nc = bacc.Bacc(trn_type="TRN2", num_devices=NCORES, debug=False)

    xT_d = nc.dram_tensor("xT", [D, N], F32R, kind="ExternalInput")
    xg_d = nc.dram_tensor("xg", [D, TB], FP32, kind="ExternalInput")
    w1_d = nc.dram_tensor("w1", [D, F], F32R, kind="ExternalInput")
    w2_d = nc.dram_tensor("w2", [D, F], F32R, kind="ExternalInput")
    w3_d = nc.dram_tensor("w3", [F, D], F32R, kind="ExternalInput")
    gw1_d = nc.dram_tensor("gw1", [D, HID], FP32, kind="ExternalInput")
    gb1_d = nc.dram_tensor("gb1", [P, NHT], FP32, kind="ExternalInput")
    gw2_d = nc.dram_tensor("gw2", [P, NHT, E], FP32, kind="ExternalInput")
    out_d = nc.dram_tensor("out_shard", [TB, D], FP32, kind="ExternalOutput")
    loss_d = nc.dram_tensor("loss", [1, 1], FP32, kind="ExternalOutput")

    xT_r = xT_d.ap().rearrange("(c p) n -> p c n", p=P)
    w1_r = w1_d.ap().rearrange("(c p) f -> p c f", p=P)
    w2_r = w2_d.ap().rearrange("(c p) f -> p c f", p=P)
    w3_r = w3_d.ap().rearrange("(t p) d -> p t d", p=P)
    gw1_r = gw1_d.ap().rearrange("(c p) h -> p c h", p=P)

    with tile.TileContext(nc) as tc, ExitStack() as ctx:
        cpool = ctx.enter_context(tc.tile_pool(name="cpool", bufs=1))
        dr = ctx.enter_context(tc.tile_pool(name="dr", bufs=1, space="DRAM"))

        ident8 = cpool.tile([8, 8], FP32, tag="ident8")
        make_identity(nc, ident8[:])
        ident128 = cpool.tile([P, P], FP32, tag="ident128")
        make_identity(nc, ident128[:])
        wall = cpool.tile([P, NB, NTS], FP32, tag="wall")

        pay_w = dr.tile([E, TB], FP32, tag="pay_w")
        pay_s = dr.tile([E, TB], FP32, tag="pay_s")
        w_a2a = dr.tile([E, TB], FP32, tag="w_a2a")
        s_ag = dr.tile([NCORES * E, TB], FP32, tag="s_ag", addr_space="Shared")
        y_full = dr.tile([N, D], FP32, tag="y_full")
        y_rs = dr.tile([TB, D], FP32, tag="y_rs")
        csum_dr = dr.tile([NCORES * E, 1], FP32, tag="csum_dr")

        # ------------------------------------------------------------------
        # Gate phase (fp32, this core's 512-token slice)
        # ------------------------------------------------------------------
        with ExitStack() as gctx:
            gsb = gctx.enter_context(tc.tile_pool(name="gsb", bufs=1))
            gps = gctx.enter_context(tc.tile_pool(name="gps", bufs=2, space="PSUM"))

            xg_sb = gsb.tile([P, DC, TB], FP32, tag="xg_sb")
            nc.sync.dma_start(xg_sb[:], xg_d.ap().rearrange("(c p) n -> p c n", p=P))
            gw1_sb = gsb.tile([P, DC, HID], FP32, tag="gw1_sb")
            nc.sync.dma_start(gw1_sb[:], gw1_r)
            gb1_sb = gsb.tile([P, NHT], FP32, tag="gb1_sb")
            nc.sync.dma_start(gb1_sb[:], gb1_d.ap())
            gw2_sb = gsb.tile([P, NHT, E], FP32, tag="gw2_sb")
            nc.sync.dma_start(gw2_sb[:], gw2_d.ap())

            g1 = gsb.tile([P, NHT, TB], FP32, tag="g1")
            for ht in range(NHT):
                psg = gps.tile([P, TB], FP32, tag="psg")
                for dc in range(DC):
                    nc.tensor.matmul(
                        psg[:], gw1_sb[:, dc, ht * P:(ht + 1) * P], xg_sb[:, dc, :],
                        start=(dc == 0), stop=(dc == DC - 1))
                nc.scalar.activation(g1[:, ht, :], psg[:], ACTF.Relu,
                                     bias=gb1_sb[:, ht:ht + 1])

            psl = gps.tile([E, TB], FP32, tag="psl", bufs=1)
            for ht in range(NHT):
                nc.tensor.matmul(psl[:], gw2_sb[:, ht, :], g1[:, ht, :],
                                 start=(ht == 0), stop=(ht == NHT - 1))
            l_sb = gsb.tile([E, TB], FP32, tag="l_sb")
            nc.vector.tensor_copy(l_sb[:], psl[:])

            # transpose logits to [tok_p, j, e]
            lt = gsb.tile([P, NTS, E], FP32, tag="lt")
            for j in range(NTS):
                ptj = gps.tile([P, E], FP32, tag="ptj", bufs=1)
                nc.tensor.transpose(ptj[:], l_sb[:, j * P:(j + 1) * P], ident8[:])
                nc.vector.tensor_copy(lt[:, j, :], ptj[:])

            sh3 = [P, NTS, E]
            m = gsb.tile([P, NTS], FP32, tag="m")
            nc.vector.tensor_reduce(m[:], lt[:], axis=AX.X, op=OP.max)
            e8 = gsb.tile(sh3, FP32, tag="e8")
            nc.vector.tensor_tensor(e8[:], lt[:], m[:].unsqueeze(-1).broadcast_to(sh3),
                                    op=OP.subtract)
            nc.scalar.activation(e8[:], e8[:], ACTF.Exp)
            z = gsb.tile([P, NTS], FP32, tag="z")
            nc.vector.tensor_reduce(z[:], e8[:], axis=AX.X, op=OP.add)
            rz = gsb.tile([P, NTS], FP32, tag="rz")
            nc.vector.reciprocal(rz[:], z[:])
            s_sc = gsb.tile(sh3, FP32, tag="s_sc")
            nc.vector.tensor_tensor(s_sc[:], e8[:],
                                    rz[:].unsqueeze(-1).broadcast_to(sh3), op=OP.mult)
            s0 = gsb.tile([P, NTS], FP32, tag="s0")
            nc.vector.tensor_reduce(s0[:], s_sc[:], axis=AX.X, op=OP.max)
            is0 = gsb.tile(sh3, FP32, tag="is0")
            nc.vector.tensor_tensor(is0[:], s_sc[:],
                                    s0[:].unsqueeze(-1).broadcast_to(sh3),
                                    op=OP.is_equal)
            sm = gsb.tile(sh3, FP32, tag="sm")
            nc.vector.scalar_tensor_tensor(sm[:], in0=is0[:], scalar=-1e30,
                                           in1=s_sc[:], op0=OP.mult, op1=OP.add)
            s1 = gsb.tile([P, NTS], FP32, tag="s1")
            nc.vector.tensor_reduce(s1[:], sm[:], axis=AX.X, op=OP.max)
            is1 = gsb.tile(sh3, FP32, tag="is1")
            nc.vector.tensor_tensor(is1[:], sm[:],
                                    s1[:].unsqueeze(-1).broadcast_to(sh3),
                                    op=OP.is_equal)
            d01 = gsb.tile([P, NTS], FP32, tag="d01")
            nc.vector.tensor_tensor(d01[:], s0[:], s1[:], op=OP.subtract)
            w0 = gsb.tile([P, NTS], FP32, tag="w0")
            nc.scalar.activation(w0[:], d01[:], ACTF.Sigmoid)
            w1v = gsb.tile([P, NTS], FP32, tag="w1v")
            nc.vector.tensor_scalar(w1v[:], w0[:], -1.0, 1.0, op0=OP.mult, op1=OP.add)
            t1 = gsb.tile(sh3, FP32, tag="t1")
            nc.vector.tensor_tensor(t1[:], is0[:],
                                    w0[:].unsqueeze(-1).broadcast_to(sh3), op=OP.mult)
            t2 = gsb.tile(sh3, FP32, tag="t2")
            nc.vector.tensor_tensor(t2[:], is1[:],
                                    w1v[:].unsqueeze(-1).broadcast_to(sh3), op=OP.mult)
            wmat = gsb.tile(sh3, FP32, tag="wmat")
            nc.vector.tensor_tensor(wmat[:], t1[:], t2[:], op=OP.add)

            # payloads [E, TB]: row e, col u=j*128+p
            pay_w_sb = gsb.tile([E, TB], FP32, tag="pay_w_sb")
            pay_s_sb = gsb.tile([E, TB], FP32, tag="pay_s_sb")
            for j in range(NTS):
                pw = gps.tile([E, P], FP32, tag="pw", bufs=1)
                nc.tensor.transpose(pw[:], wmat[:, j, :], ident128[:])
                nc.vector.tensor_copy(pay_w_sb[:, j * P:(j + 1) * P], pw[:])
                psc = gps.tile([E, P], FP32, tag="psc", bufs=1)
                nc.tensor.transpose(psc[:], s_sc[:, j, :], ident128[:])
                nc.vector.tensor_copy(pay_s_sb[:, j * P:(j + 1) * P], psc[:])
            nc.sync.dma_start(pay_w[:], pay_w_sb[:])
            nc.sync.dma_start(pay_s[:], pay_s_sb[:])

            nc.gpsimd.collective_compute("AllToAll", OP.bypass, replica_groups=RG,
                                         ins=[pay_w[:]], outs=[w_a2a[:]])
            nc.gpsimd.collective_compute("AllGather", OP.bypass, replica_groups=RG,
                                         ins=[pay_s[:]], outs=[s_ag[:]])

            # wall[p, b, s] = weight of my expert for global token b*512 + s*128 + p
            nc.sync.dma_start(wall[:], w_a2a[:].rearrange("e (s p) -> p e s", p=P))

            # balance loss (replicated on every core; graded from core 0)
            sag_sb = gsb.tile([NCORES * E, TB], FP32, tag="sag_sb")
            nc.sync.dma_start(sag_sb[:], s_ag[:])
            csum = gsb.tile([NCORES * E, 1], FP32, tag="csum")
            nc.vector.tensor_reduce(csum[:], sag_sb[:], axis=AX.X, op=OP.add)
            nc.sync.dma_start(csum_dr[:], csum[:])
            row = gsb.tile([1, NCORES * E], FP32, tag="row")
            nc.sync.dma_start(row[:], csum_dr[:].rearrange("a b -> b a"))
            m8 = gsb.tile([1, E], FP32, tag="m8")
            nc.vector.tensor_reduce(
                m8[:], row[:].rearrange("p (r e) -> p e r", e=E), axis=AX.X, op=OP.add)
            nc.vector.tensor_scalar(m8[:], m8[:], 1.0 / N, None, op0=OP.mult)
            mp8 = gsb.tile([1, E], FP32, tag="mp8")
            nc.vector.tensor_scalar(mp8[:], m8[:], 1e-8, None, op0=OP.add)
            ln8 = gsb.tile([1, E], FP32, tag="ln8")
            nc.scalar.activation(ln8[:], mp8[:], ACTF.Ln)
            pr8 = gsb.tile([1, E], FP32, tag="pr8")
            nc.vector.tensor_tensor(pr8[:], m8[:], ln8[:], op=OP.mult)
            tot = gsb.tile([1, 1], FP32, tag="tot")
            nc.vector.tensor_reduce(tot[:], pr8[:], axis=AX.X, op=OP.add)
            nc.vector.tensor_scalar(tot[:], tot[:], float(E), None, op0=OP.mult)
            nc.sync.dma_start(loss_d.ap(), tot[:])

        # ------------------------------------------------------------------
        # Expert phase (f32r, dense over all 4096 tokens)
        # ------------------------------------------------------------------
        xbp = ctx.enter_context(tc.tile_pool(name="xbp", bufs=2))
        wp = ctx.enter_context(tc.tile_pool(name="wp", bufs=2))
        w3p = ctx.enter_context(tc.tile_pool(name="w3p", bufs=4))
        hp = ctx.enter_context(tc.tile_pool(name="hp", bufs=34))
        yp = ctx.enter_context(tc.tile_pool(name="yp", bufs=4))
        php = ctx.enter_context(tc.tile_pool(name="php", bufs=4, space="PSUM"))
        pyp = ctx.enter_context(tc.tile_pool(name="pyp", bufs=4, space="PSUM"))

        for b in range(NB):
            xb = xbp.tile([P, DC, TB], F32R, tag="xb")
            nc.sync.dma_start(xb[:], xT_r[:, :, b * TB:(b + 1) * TB])

            h_tiles = []
            for fg in range(NFG):
                w1t = wp.tile([P, DC, 512], F32R, tag="w1t")
                nc.sync.dma_start(w1t[:], w1_r[:, :, fg * 512:(fg + 1) * 512])
                w2t = wp.tile([P, DC, 512], F32R, tag="w2t")
                nc.sync.dma_start(w2t[:], w2_r[:, :, fg * 512:(fg + 1) * 512])
                for fs in range(4):
                    ph1 = php.tile([P, TB], FP32, tag="ph")
                    for dc in range(DC):
                        nc.tensor.matmul(
                            ph1[:], w1t[:, dc, fs * P:(fs + 1) * P], xb[:, dc, :],
                            start=(dc == 0), stop=(dc == DC - 1))
                    ph2 = php.tile([P, TB], FP32, tag="ph")
                    for dc in range(DC):
                        nc.tensor.matmul(
                            ph2[:], w2t[:, dc, fs * P:(fs + 1) * P], xb[:, dc, :],
                            start=(dc == 0), stop=(dc == DC - 1))
                    hsl = hp.tile([P, TB], F32R, tag="h")
                    nc.scalar.activation(hsl[:], ph1[:], ACTF.Silu)
                    nc.vector.tensor_tensor(hsl[:], hsl[:], ph2[:], op=OP.mult)
                    h_tiles.append(hsl)

            for dg in range(2):
                pys = [pyp.tile([P, 512], FP32, tag="py", name=f"py{b}_{dg}_{i}")
                       for i in range(NTS)]
                for ft in range(F // P):
                    w3t = w3p.tile([P, 512], F32R, tag="w3t")
                    nc.sync.dma_start(w3t[:], w3_r[:, ft, dg * 512:(dg + 1) * 512])
                    h_r = h_tiles[ft]
                    for ts in range(NTS):
                        nc.tensor.matmul(
                            pys[ts][:], h_r[:, ts * P:(ts + 1) * P], w3t[:],
                            start=(ft == 0), stop=(ft == F // P - 1))
                for ts in range(NTS):
                    ysb = yp.tile([P, 512], FP32, tag="ysb")
                    nc.vector.tensor_scalar(ysb[:], pys[ts][:],
                                            wall[:, b, ts].unsqueeze(-1), None,
                                            op0=OP.mult)
                    nc.sync.dma_start(
                        y_full[b * TB + ts * P: b * TB + (ts + 1) * P,
                               dg * 512:(dg + 1) * 512], ysb[:])

        nc.gpsimd.collective_compute("ReduceScatter", OP.add, replica_groups=RG,
                                     ins=[y_full[:]], outs=[y_rs[:]])
        nc.sync.dma_start(out_d.ap(), y_rs[:])

    nc.compile()
    return nc


def kernel(x, gw1, gb1, gw2, W1, W2, W3):
    global _CACHED_NC
    x = np.asarray(x, dtype=np.float32)
    gw1 = np.asarray(gw1, dtype=np.float32)
    gb1 = np.asarray(gb1, dtype=np.float32)
    gw2 = np.asarray(gw2, dtype=np.float32)
    W1 = np.asarray(W1, dtype=np.float32)
    W2 = np.asarray(W2, dtype=np.float32)
    W3 = np.asarray(W3, dtype=np.float32)

    xT = np.ascontiguousarray(x.reshape(N, D).T)                 # [D, N]
    gb1_h = np.ascontiguousarray(gb1.reshape(NHT, P).T)          # [128, 4]
    gw2_h = np.ascontiguousarray(gw2.reshape(NHT, P, E).transpose(1, 0, 2))

    if _CACHED_NC is None:
        _CACHED_NC = build()
    nc = _CACHED_NC

    in_maps = []
    for c in range(NCORES):
        in_maps.append({
            "xT": xT,
            "xg": np.ascontiguousarray(xT[:, c * TB:(c + 1) * TB]),
            "w1": np.ascontiguousarray(W1[c]),
            "w2": np.ascontiguousarray(W2[c]),
            "w3": np.ascontiguousarray(W3[c]),
            "gw1": gw1,
            "gb1": gb1_h,
            "gw2": gw2_h,
        })
    res = bass_utils.run_bass_kernel_spmd(nc, in_maps, core_ids=list(range(NCORES)))
    out = np.concatenate([res.results[c]["out_shard"] for c in range(NCORES)],
                         axis=0).reshape(B, S, D)
    loss = np.float32(res.results[0]["loss"][0, 0])
    return out, loss


# revision 5
# speedup vs baseline: 1359.2501x; 1359.2501x over previous
"""MoE layer (B=2, S=2048, D=1024, F=4096, E=8, top-2) on 8 Trainium2 NeuronCores.

Strategy: expert-parallel DENSE. Core c holds expert c's weights and computes
expert c's SwiGLU output for ALL 4096 tokens (perfect load balance regardless of
routing skew). The gate network runs fp32 (exact routing), sharded over tokens
(512 per core); per-expert combine weights (zero except each token's top-2) are
exchanged with a tiny AllToAll, each core scales its expert output rows by its
weight column, and a ReduceScatter sums the 8 weighted expert outputs and leaves
each core with its 512-token shard of the final output. The host concatenates
the 8 shards. Expert matmuls run in float32r (TF32-like, ~1.5e-4 rel err, full
PE rate); the gate runs in true fp32 (4x slower, but tiny).
"""
import numpy as np
from contextlib import ExitStack

import concourse.bass as bass
from concourse import bacc
import concourse.mybir as mybir
import concourse.tile as tile
import concourse.bass_utils as bass_utils
from concourse.masks import make_identity

P = 128
D, F, E, HID = 1024, 4096, 8, 512
NCORES = 8
B, S = 2, 2048
N = B * S            # 4096 tokens
TB = 512             # tokens per block == tokens per core gate-slice
NB = N // TB         # 8
DC = D // P          # 8 contraction chunks over D
NFG = F // 512       # 8 f-groups of 512
NTS = TB // P        # 4 token subtiles
NHT = HID // P       # 4 hidden tiles (gate)
FP32 = mybir.dt.float32
F32R = mybir.dt.float32r
AX = mybir.AxisListType
OP = mybir.AluOpType
ACTF = mybir.ActivationFunctionType
RG = [list(range(NCORES))]

_CACHED_NC = None


def build():
    nc = bacc.Bacc(trn_type="TRN2", num_devices=NCORES, debug=False)

    xT_d = nc.dram_tensor("xT", [D, N], F32R, kind="ExternalInput")
    xg_d = nc.dram_tensor("xg", [D, TB], FP32, kind="ExternalInput")
    w1_d = nc.dram_tensor("w1", [D, F], F32R, kind="ExternalInput")
    w2_d = nc.dram_tensor("w2", [D, F], F32R, kind="ExternalInput")
    w3_d = nc.dram_tensor("w3", [F, D], F32R, kind="ExternalInput")
    gw1_d = nc.dram_tensor("gw1", [D, HID], FP32, kind="ExternalInput")
    gb1_d = nc.dram_tensor("gb1", [P, NHT], FP32, kind="ExternalInput")
    gw2_d = nc.dram_tensor("gw2", [P, NHT, E], FP32, kind="ExternalInput")
    out_d = nc.dram_tensor("out_shard", [TB, D], FP32, kind="ExternalOutput")
    loss_d = nc.dram_tensor("loss", [1, 1], FP32, kind="ExternalOutput")

    xT_r = xT_d.ap().rearrange("(c p) n -> p c n", p=P)
    w1_r = w1_d.ap().rearrange("(c p) f -> p c f", p=P)
    w2_r = w2_d.ap().rearrange("(c p) f -> p c f", p=P)
    w3_r = w3_d.ap().rearrange("(t p) d -> p t d", p=P)
    gw1_r = gw1_d.ap().rearrange("(c p) h -> p c h", p=P)

    with tile.TileContext(nc) as tc, ExitStack() as ctx:
        cpool = ctx.enter_context(tc.tile_pool(name="cpool", bufs=1))
        dr = ctx.enter_context(tc.tile_pool(name="dr", bufs=1, space="DRAM"))

        ident8 = cpool.tile([8, 8], FP32, tag="ident8")
        make_identity(nc, ident8[:])
        ident128 = cpool.tile([P, P], FP32, tag="ident128")
        make_identity(nc, ident128[:])
        wall = cpool.tile([P, NB, NTS], FP32, tag="wall")

        pay_w = dr.tile([E, TB], FP32, tag="pay_w")
        pay_s = dr.tile([E, TB], FP32, tag="pay_s")
        w_a2a = dr.tile([E, TB], FP32, tag="w_a2a")
        s_ag = dr.tile([NCORES * E, TB], FP32, tag="s_ag", addr_space="Shared")
        y_full = dr.tile([N, D], FP32, tag="y_full")
        y_rs = dr.tile([TB, D], FP32, tag="y_rs")
        csum_dr = dr.tile([NCORES * E, 1], FP32, tag="csum_dr")

        # ------------------------------------------------------------------
        # Gate phase (fp32, this core's 512-token slice)
        # ------------------------------------------------------------------
        with ExitStack() as gctx:
            gsb = gctx.enter_context(tc.tile_pool(name="gsb", bufs=1))
            gps = gctx.enter_context(tc.tile_pool(name="gps", bufs=2, space="PSUM"))

            xg_sb = gsb.tile([P, DC, TB], FP32, tag="xg_sb")
            nc.sync.dma_start(xg_sb[:], xg_d.ap().rearrange("(c p) n -> p c n", p=P))
            gw1_sb = gsb.tile([P, DC, HID], FP32, tag="gw1_sb")
            nc.sync.dma_start(gw1_sb[:], gw1_r)
            gb1_sb = gsb.tile([P, NHT], FP32, tag="gb1_sb")
            nc.sync.dma_start(gb1_sb[:], gb1_d.ap())
            gw2_sb = gsb.tile([P, NHT, E], FP32, tag="gw2_sb")
            nc.sync.dma_start(gw2_sb[:], gw2_d.ap())

            g1 = gsb.tile([P, NHT, TB], FP32, tag="g1")
            for ht in range(NHT):
                psg = gps.tile([P, TB], FP32, tag="psg")
                for dc in range(DC):
                    nc.tensor.matmul(
                        psg[:], gw1_sb[:, dc, ht * P:(ht + 1) * P], xg_sb[:, dc, :],
                        start=(dc == 0), stop=(dc == DC - 1))
                nc.scalar.activation(g1[:, ht, :], psg[:], ACTF.Relu,
                                     bias=gb1_sb[:, ht:ht + 1])

            psl = gps.tile([E, TB], FP32, tag="psl", bufs=1)
            for ht in range(NHT):
                nc.tensor.matmul(psl[:], gw2_sb[:, ht, :], g1[:, ht, :],
                                 start=(ht == 0), stop=(ht == NHT - 1))
            l_sb = gsb.tile([E, TB], FP32, tag="l_sb")
            nc.vector.tensor_copy(l_sb[:], psl[:])

            # transpose logits to [tok_p, j, e]
            lt = gsb.tile([P, NTS, E], FP32, tag="lt")
            for j in range(NTS):
                ptj = gps.tile([P, E], FP32, tag="ptj", bufs=1)
                nc.tensor.transpose(ptj[:], l_sb[:, j * P:(j + 1) * P], ident8[:])
                nc.vector.tensor_copy(lt[:, j, :], ptj[:])

            sh3 = [P, NTS, E]
            m = gsb.tile([P, NTS], FP32, tag="m")
            nc.vector.tensor_reduce(m[:], lt[:], axis=AX.X, op=OP.max)
            e8 = gsb.tile(sh3, FP32, tag="e8")
            nc.vector.tensor_tensor(e8[:], lt[:], m[:].unsqueeze(-1).broadcast_to(sh3),
                                    op=OP.subtract)
            nc.scalar.activation(e8[:], e8[:], ACTF.Exp)
            z = gsb.tile([P, NTS], FP32, tag="z")
            nc.vector.tensor_reduce(z[:], e8[:], axis=AX.X, op=OP.add)
            rz = gsb.tile([P, NTS], FP32, tag="rz")
            nc.vector.reciprocal(rz[:], z[:])
            s_sc = gsb.tile(sh3, FP32, tag="s_sc")
            nc.vector.tensor_tensor(s_sc[:], e8[:],
                                    rz[:].unsqueeze(-1).broadcast_to(sh3), op=OP.mult)
            s0 = gsb.tile([P, NTS], FP32, tag="s0")
            nc.vector.tensor_reduce(s0[:], s_sc[:], axis=AX.X, op=OP.max)
            is0 = gsb.tile(sh3, FP32, tag="is0")
            nc.vector.tensor_tensor(is0[:], s_sc[:],
                                    s0[:].unsqueeze(-1).broadcast_to(sh3),
                                    op=OP.is_equal)
            sm = gsb.tile(sh3, FP32, tag="sm")
            nc.vector.scalar_tensor_tensor(sm[:], in0=is0[:], scalar=-1e30,
                                           in1=s_sc[:], op0=OP.mult, op1=OP.add)
            s1 = gsb.tile([P, NTS], FP32, tag="s1")
            nc.vector.tensor_reduce(s1[:], sm[:], axis=AX.X, op=OP.max)
            is1 = gsb.tile(sh3, FP32, tag="is1")
            nc.vector.tensor_tensor(is1[:], sm[:],
                                    s1[:].unsqueeze(-1).broadcast_to(sh3),
                                    op=OP.is_equal)
            d01 = gsb.tile([P, NTS], FP32, tag="d01")
            nc.vector.tensor_tensor(d01[:], s0[:], s1[:], op=OP.subtract)
            w0 = gsb.tile([P, NTS], FP32, tag="w0")
            nc.scalar.activation(w0[:], d01[:], ACTF.Sigmoid)
            w1v = gsb.tile([P, NTS], FP32, tag="w1v")
            nc.vector.tensor_scalar(w1v[:], w0[:], -1.0, 1.0, op0=OP.mult, op1=OP.add)
            t1 = gsb.tile(sh3, FP32, tag="t1")
            nc.vector.tensor_tensor(t1[:], is0[:],
                                    w0[:].unsqueeze(-1).broadcast_to(sh3), op=OP.mult)
            t2 = gsb.tile(sh3, FP32, tag="t2")
            nc.vector.tensor_tensor(t2[:], is1[:],
                                    w1v[:].unsqueeze(-1).broadcast_to(sh3), op=OP.mult)
            wmat = gsb.tile(sh3, FP32, tag="wmat")
            nc.vector.tensor_tensor(wmat[:], t1[:], t2[:], op=OP.add)

            # payloads [E, TB]: row e, col u=j*128+p
            pay_w_sb = gsb.tile([E, TB], FP32, tag="pay_w_sb")
            pay_s_sb = gsb.tile([E, TB], FP32, tag="pay_s_sb")
            for j in range(NTS):
                pw = gps.tile([E, P], FP32, tag="pw", bufs=1)
                nc.tensor.transpose(pw[:], wmat[:, j, :], ident128[:])
                nc.vector.tensor_copy(pay_w_sb[:, j * P:(j + 1) * P], pw[:])
                psc = gps.tile([E, P], FP32, tag="psc", bufs=1)
                nc.tensor.transpose(psc[:], s_sc[:, j, :], ident128[:])
                nc.vector.tensor_copy(pay_s_sb[:, j * P:(j + 1) * P], psc[:])
            nc.sync.dma_start(pay_w[:], pay_w_sb[:])
            nc.sync.dma_start(pay_s[:], pay_s_sb[:])

            nc.gpsimd.collective_compute("AllToAll", OP.bypass, replica_groups=RG,
                                         ins=[pay_w[:]], outs=[w_a2a[:]])
            nc.gpsimd.collective_compute("AllGather", OP.bypass, replica_groups=RG,
                                         ins=[pay_s[:]], outs=[s_ag[:]])

            # wall[p, b, s] = weight of my expert for global token b*512 + s*128 + p
            nc.sync.dma_start(wall[:], w_a2a[:].rearrange("e (s p) -> p e s", p=P))

            # balance loss (replicated on every core; graded from core 0)
            sag_sb = gsb.tile([NCORES * E, TB], FP32, tag="sag_sb")
            nc.sync.dma_start(sag_sb[:], s_ag[:])
            csum = gsb.tile([NCORES * E, 1], FP32, tag="csum")
            nc.vector.tensor_reduce(csum[:], sag_sb[:], axis=AX.X, op=OP.add)
            nc.sync.dma_start(csum_dr[:], csum[:])
            row = gsb.tile([1, NCORES * E], FP32, tag="row")
            nc.sync.dma_start(row[:], csum_dr[:].rearrange("a b -> b a"))
            m8 = gsb.tile([1, E], FP32, tag="m8")
            nc.vector.tensor_reduce(
                m8[:], row[:].rearrange("p (r e) -> p e r", e=E), axis=AX.X, op=OP.add)
            nc.vector.tensor_scalar(m8[:], m8[:], 1.0 / N, None, op0=OP.mult)
            mp8 = gsb.tile([1, E], FP32, tag="mp8")
            nc.vector.tensor_scalar(mp8[:], m8[:], 1e-8, None, op0=OP.add)
            ln8 = gsb.tile([1, E], FP32, tag="ln8")
            nc.scalar.activation(ln8[:], mp8[:], ACTF.Ln)
            pr8 = gsb.tile([1, E], FP32, tag="pr8")
            nc.vector.tensor_tensor(pr8[:], m8[:], ln8[:], op=OP.mult)
            tot = gsb.tile([1, 1], FP32, tag="tot")
            nc.vector.tensor_reduce(tot[:], pr8[:], axis=AX.X, op=OP.add)
            nc.vector.tensor_scalar(tot[:], tot[:], float(E), None, op0=OP.mult)
            nc.sync.dma_start(loss_d.ap(), tot[:])

        # ------------------------------------------------------------------
        # Expert phase (f32r, dense over all 4096 tokens)
        # ------------------------------------------------------------------
        xbp = ctx.enter_context(tc.tile_pool(name="xbp", bufs=2))
        wp = ctx.enter_context(tc.tile_pool(name="wp", bufs=2))
        w3p = ctx.enter_context(tc.tile_pool(name="w3p", bufs=4))
        hp = ctx.enter_context(tc.tile_pool(name="hp", bufs=34))
        yp = ctx.enter_context(tc.tile_pool(name="yp", bufs=4))
        php = ctx.enter_context(tc.tile_pool(name="php", bufs=4, space="PSUM"))
        pyp = ctx.enter_context(tc.tile_pool(name="pyp", bufs=4, space="PSUM"))

        for b in range(NB):
            xb = xbp.tile([P, DC, TB], F32R, tag="xb")
            nc.sync.dma_start(xb[:], xT_r[:, :, b * TB:(b + 1) * TB])

            h_tiles = []
            for fg in range(NFG):
                w1t = wp.tile([P, DC, 512], F32R, tag="w1t")
                nc.sync.dma_start(w1t[:], w1_r[:, :, fg * 512:(fg + 1) * 512])
                w2t = wp.tile([P, DC, 512], F32R, tag="w2t")
                nc.sync.dma_start(w2t[:], w2_r[:, :, fg * 512:(fg + 1) * 512])
                for fs in range(4):
                    ph1 = php.tile([P, TB], FP32, tag="ph")
                    for dc in range(DC):
                        nc.tensor.matmul(
                            ph1[:], w1t[:, dc, fs * P:(fs + 1) * P], xb[:, dc, :],
                            start=(dc == 0), stop=(dc == DC - 1))
                    ph2 = php.tile([P, TB], FP32, tag="ph")
                    for dc in range(DC):
                        nc.tensor.matmul(
                            ph2[:], w2t[:, dc, fs * P:(fs + 1) * P], xb[:, dc, :],
                            start=(dc == 0), stop=(dc == DC - 1))
                    hsl = hp.tile([P, TB], F32R, tag="h")
                    nc.scalar.activation(hsl[:], ph1[:], ACTF.Silu)
                    nc.vector.tensor_tensor(hsl[:], hsl[:], ph2[:], op=OP.mult)
                    h_tiles.append(hsl)

            for dg in range(2):
                pys = [pyp.tile([P, 512], FP32, tag="py", name=f"py{b}_{dg}_{i}")
                       for i in range(NTS)]
                for ft in range(F // P):
                    w3t = w3p.tile([P, 512], F32R, tag="w3t")
                    nc.sync.dma_start(w3t[:], w3_r[:, ft, dg * 512:(dg + 1) * 512])
                    h_r = h_tiles[ft]
                    for ts in range(NTS):
                        nc.tensor.matmul(
                            pys[ts][:], h_r[:, ts * P:(ts + 1) * P], w3t[:],
                            start=(ft == 0), stop=(ft == F // P - 1))
                for ts in range(NTS):
                    ysb = yp.tile([P, 512], FP32, tag="ysb")
                    nc.vector.tensor_scalar(ysb[:], pys[ts][:],
                                            wall[:, b, ts].unsqueeze(-1), None,
                                            op0=OP.mult)
                    nc.sync.dma_start(
                        y_full[b * TB + ts * P: b * TB + (ts + 1) * P,
                               dg * 512:(dg + 1) * 512], ysb[:])

        nc.gpsimd.collective_compute("ReduceScatter", OP.add, replica_groups=RG,
                                     ins=[y_full[:]], outs=[y_rs[:]])
        nc.sync.dma_start(out_d.ap(), y_rs[:])

    nc.compile()
    return nc


class Runner:
    """Builds the Bass module once and caches a jitted PJRT executable, so
    repeated kernel() calls skip Tile scheduling / bacc / walrus / XLA compile.
    Mirrors concourse.bass2jax.run_bass_via_pjrt."""

    def __init__(self):
        import jax
        from jax.sharding import Mesh, PartitionSpec
        from jax.experimental.shard_map import shard_map
        from concourse import bass2jax

        bass2jax.install_neuronx_cc_hook()
        nc = build()
        self.nc = nc

        partition_name = (nc.partition_id_tensor.name
                          if nc.partition_id_tensor else None)
        in_names, out_names, out_avals, zero_outs = [], [], [], []
        for alloc in nc.m.functions[0].allocations:
            if not isinstance(alloc, mybir.MemoryLocationSet):
                continue
            name = alloc.memorylocations[0].name
            if alloc.kind == "ExternalInput":
                if name != partition_name:
                    in_names.append(name)
            elif alloc.kind == "ExternalOutput":
                out_names.append(name)
                shape = tuple(alloc.tensor_shape)
                dtype = mybir.dt.np(alloc.dtype)
                out_avals.append(jax.core.ShapedArray(shape, dtype))
                zero_outs.append(np.zeros(shape, dtype))
        self.in_names = list(in_names)
        self.out_names = out_names
        self.out_avals = out_avals
        self.zero_outs = zero_outs
        n_params = len(in_names)
        n_outs = len(out_avals)
        all_in_names = list(in_names) + list(out_names)
        if partition_name is not None:
            all_in_names.append(partition_name)

        def _body(*args):
            operands = list(args)
            if partition_name is not None:
                operands.append(bass2jax.partition_id_tensor())
            outs = bass2jax._bass_exec_p.bind(
                *operands,
                out_avals=tuple(out_avals),
                in_names=tuple(all_in_names),
                out_names=tuple(out_names),
                lowering_input_output_aliases=(),
                sim_require_finite=True,
                sim_require_nnan=True,
                nc=nc,
            )
            return tuple(outs)

        donate = tuple(range(n_params, n_params + n_outs))
        devices = jax.devices()[:NCORES]
        self.mesh = Mesh(np.asarray(devices), ("core",))
        in_specs = (PartitionSpec("core"),) * (n_params + n_outs)
        out_specs = (PartitionSpec("core"),) * n_outs
        self.fn = jax.jit(
            shard_map(_body, mesh=self.mesh, in_specs=in_specs,
                      out_specs=out_specs, check_rep=False),
            donate_argnums=donate, keep_unused=True)
        self.n_params = n_params

    def concat_inputs(self, in_maps):
        return [np.concatenate([np.asarray(in_maps[c][nm])
                                for c in range(NCORES)], axis=0)
                for nm in self.in_names]

    def zeros(self):
        return [np.zeros((NCORES * z.shape[0], *z.shape[1:]), z.dtype)
                for z in self.zero_outs]

    def run(self, concat_in, concat_zeros=None):
        if concat_zeros is None:
            concat_zeros = self.zeros()
        out_arrs = self.fn(*concat_in, *concat_zeros)
        return [
            {nm: np.asarray(out_arrs[i]).reshape(NCORES, *self.out_avals[i].shape)[c]
             for i, nm in enumerate(self.out_names)}
            for c in range(NCORES)
        ]


_RUNNER = None


def get_runner():
    global _RUNNER
    if _RUNNER is None:
        _RUNNER = Runner()
    return _RUNNER


def make_in_maps(x, gw1, gb1, gw2, W1, W2, W3):
    xT = np.ascontiguousarray(x.reshape(N, D).T)                 # [D, N]
    gb1_h = np.ascontiguousarray(gb1.reshape(NHT, P).T)          # [128, 4]
    gw2_h = np.ascontiguousarray(gw2.reshape(NHT, P, E).transpose(1, 0, 2))
    in_maps = []
    for c in range(NCORES):
        in_maps.append({
            "xT": xT,
            "xg": np.ascontiguousarray(xT[:, c * TB:(c + 1) * TB]),
            "w1": np.ascontiguousarray(W1[c]),
            "w2": np.ascontiguousarray(W2[c]),
            "w3": np.ascontiguousarray(W3[c]),
            "gw1": gw1,
            "gb1": gb1_h,
            "gw2": gw2_h,
        })
    return in_maps


def kernel(x, gw1, gb1, gw2, W1, W2, W3):
    x = np.asarray(x, dtype=np.float32)
    gw1 = np.asarray(gw1, dtype=np.float32)
    gb1 = np.asarray(gb1, dtype=np.float32)
    gw2 = np.asarray(gw2, dtype=np.float32)
    W1 = np.asarray(W1, dtype=np.float32)
    W2 = np.asarray(W2, dtype=np.float32)
    W3 = np.asarray(W3, dtype=np.float32)

    runner = get_runner()
    in_maps = make_in_maps(x, gw1, gb1, gw2, W1, W2, W3)
    results = runner.run(runner.concat_inputs(in_maps))
    out = np.concatenate([results[c]["out_shard"] for c in range(NCORES)],
                         axis=0).reshape(B, S, D)
    loss = np.float32(results[0]["loss"][0, 0])
    return out, loss
